# revision 42
# baseline (speedup 1.0000x reference)
"""Multi-head attention Trainium2 kernel, 8-core batch+head sharded.

Sharding: cores 0-3 -> batch 0, cores 4-7 -> batch 1; each core computes 4
heads. Host compacts queries by q_mask and keys by v_mask (masked softmax
over the kept key subset equals the reference's additive-mask softmax),
transposes/packs inputs, and sums the 4 per-core partial output projections
per batch (the row-sharded-Wo "all-reduce"), adds bo, scatters rows back.

v2 schedule: the ScalarE exp stream (~50us) is the pacer, so everything
else is arranged to hide beneath it.  Blocks are processed pair-major
(2 heads at a time) so PSUM fits in 8 banks with a spare projection bank:
K/V/Q projections and the output projection interleave into the exp
stream's PE slack.  ScalarE does exp ONLY (no copies); broadcast of 1/Z
goes via DMA, evacuations via VectorE, reciprocal via the fast DVE approx.

Self-contained: hardcodes B=2,S=2048,D=1024,H=16,HS=64,OUT=1024.
"""
import sys, types

sys.path.insert(0, '/opt/trn_rl_repo')

# ---- NTFF profile hook (image's antenv lacks axon_hooks) ----
if "antenv.axon_hooks" not in sys.modules:
    _hook_mod = types.ModuleType("antenv.axon_hooks")
    _hook_mod._hook = None
    def _set_hook(h, _m=_hook_mod):
        _m._hook = h
    def _get_hook(_m=_hook_mod):
        return _m._hook
    _hook_mod.set_axon_ntff_profile_hook = _set_hook
    _hook_mod.get_axon_ntff_profile_hook = _get_hook
    sys.modules["antenv.axon_hooks"] = _hook_mod
    try:
        from trn_agent_boot.trn_boot import _ntff_profile_via_ctypes
        _set_hook(_ntff_profile_via_ctypes('/opt/axon/libaxon_pjrt.so'))
    except Exception:
        pass

import numpy as np
import ml_dtypes
import concourse.bass as bass
import concourse.tile as tile
import concourse.mybir as mybir
from concourse import bass_utils, bacc

B, S, D, H, HS, OUT = 2, 2048, 1024, 16, 64, 1024
HPC = 4          # heads per core
NCORES = 8
DT = D // 128    # 8 d-tiles
F32 = mybir.dt.float32
F16 = mybir.dt.float16
SCALE = float(1.0 / np.sqrt(HS))
KPAD_BIAS = -1e5  # exp underflows to exactly 0.0


def _bblocks(total):
    """(off, len) 512-wide blocks + remainder (PSUM-bank-sized)."""
    out = []
    off = 0
    while off < total:
        w = min(512, total - off)
        out.append((off, w))
        off += w
    return out


def build_kernel(SQP, SKP):
    """One SPMD Bass program. SQP/SKP: padded (mult of 128) query/key counts."""
    SKT = SKP // 128
    QBL = _bblocks(SQP)      # q blocks
    NB = len(QBL)
    EXPF = mybir.ActivationFunctionType.Exp
    nc = bacc.Bacc("TRN2", target_bir_lowering=False, debug=False,
                   num_devices=NCORES)

    # x tensors arrive chunk-major, partition-major: [128, DT, cw] per chunk
    # so each chunk DMA moves DT*cw*2 contiguous bytes per partition
    # (DMA cost is packet-count-bound; big per-partition lines = full BW).
    KC = []
    off = 0
    while off < SKP:
        w = min(256, SKP - off)
        KC.append((off, w))
        off += w
    xq_d = [nc.dram_tensor(f'xq{i}', [128, DT, blen], F16,
                           kind='ExternalInput').ap()
            for i, (b0, blen) in enumerate(_bblocks(SQP))]
    xk_d = [nc.dram_tensor(f'xk{i}', [128, DT, clen], F16,
                           kind='ExternalInput').ap()
            for i, (c0, clen) in enumerate(KC)]
    xv_d = [nc.dram_tensor(f'xv{s}', [128, DT, 128], F16,
                           kind='ExternalInput').ap()
            for s in range(SKP // 128)]
    wq_d = [nc.dram_tensor(f'wq{p}', [128, DT, 128], F16,
                           kind='ExternalInput').ap() for p in range(2)]
    wk_d = [nc.dram_tensor(f'wk{p}', [128, DT, 128], F16,
                           kind='ExternalInput').ap() for p in range(2)]
    wv_d = nc.dram_tensor('wv', [128, DT, 256], F16, kind='ExternalInput').ap()
    wo_d = nc.dram_tensor('wo', [128, 2, OUT], F16, kind='ExternalInput').ap()
    qkb_d = nc.dram_tensor('qkb', [128, 4], F32, kind='ExternalInput').ap()
    vb_d = nc.dram_tensor('vb', [1, 256], F32, kind='ExternalInput').ap()
    kbias_d = nc.dram_tensor('kbias', [128, SKT], F32, kind='ExternalInput').ap()
    outp = nc.dram_tensor('outp', [SQP, OUT], F16, kind='ExternalOutput').ap()

    with tile.TileContext(nc) as tc, \
         nc.allow_low_precision(reason="f16 activations are within tolerance"):
        with tc.tile_pool(name="const", bufs=1) as constp, \
             tc.tile_pool(name="persist", bufs=1) as persist, \
             tc.tile_pool(name="etile", bufs=22) as etile, \
             tc.tile_pool(name="work", bufs=2) as work, \

             tc.tile_pool(name="ps_s2", bufs=2, space="PSUM") as ps_s2, \
             tc.tile_pool(name="ps_acc", bufs=2, space="PSUM") as ps_acc, \
             tc.tile_pool(name="ps_zp", bufs=1, space="PSUM") as ps_zp, \
             tc.tile_pool(name="ps_pj", bufs=1, space="PSUM") as ps_pj:

            # ---- constants / weights ----
            wq_sb = [constp.tile([128, DT, 128], F16, name=f"wq{p}")
                     for p in range(2)]
            wk_sb = [constp.tile([128, DT, 128], F16, name=f"wk{p}")
                     for p in range(2)]
            wv_sb = constp.tile([128, DT, 256], F16)
            wo_sb = constp.tile([128, 2, OUT], F16)
            qkb_sb = constp.tile([128, 4], F32)
            vb_bc = constp.tile([128, 256], F32)
            kbias_sb = constp.tile([128, SKT], F32)
            ones_f = constp.tile([128, 64], F32)
            ones_h = constp.tile([128, 64], F16)

            # ---- persistent activations ----
            # xq/xk as one tile per chunk: chunk DMAs then have 4KB
            # contiguous per partition on BOTH sides (big DMA packets).
            xq_sb = [persist.tile([128, DT, blen], F16, name=f"xq{i}")
                     for i, (b0, blen) in enumerate(QBL)]
            xk_sb = [persist.tile([128, DT, clen], F16, name=f"xk{i}")
                     for i, (c0, clen) in enumerate(KC)]
            xv_sb = [persist.tile([128, DT, 128], F16, name=f"xv{s}")
                     for s in range(SKT)]
            qt_sb = persist.tile([128, 2, SQP], F16)   # [:, pair, :] Q^T, 2 heads stacked
            kt_sb = persist.tile([128, 2, SKP], F16)
            v_sb = persist.tile([128, SKT, 256], F16)  # V natural, 4 heads
            ot_sb = persist.tile([128, 2, SQP], F16)   # normalized O^T (outproj lhsT)
            zq_sb = persist.tile([128, SQP], F32)
            zinv_sb = persist.tile([128, SQP], F32)

            # ALL input on ONE queue in strict need order: the DMA engines
            # are a shared packet FIFO, so a second active queue halves the
            # critical stream's arrival rate.  Tiny tensors first (a 16B
            # DMA behind bulk traffic arrives ~10us late).
            nc.sync.dma_start(out=qkb_sb, in_=qkb_d)
            nc.sync.dma_start(out=kbias_sb, in_=kbias_d)
            nc.sync.dma_start(out=vb_bc, in_=bass.AP(
                tensor=vb_d.tensor, offset=vb_d.offset,
                ap=[[0, 128], vb_d.ap[1]]))
            nc.sync.dma_start(out=wk_sb[0], in_=wk_d[0])
            nc.sync.dma_start(out=xk_sb[0], in_=xk_d[0])
            nc.sync.dma_start(out=wq_sb[0], in_=wq_d[0])
            nc.sync.dma_start(out=xq_sb[0], in_=xq_d[0])
            nc.sync.dma_start(out=wv_sb, in_=wv_d)
            # interleave the per-skt xv chunks with the remaining K-side
            # loads: both streams arrive just-in-time for their consumers.
            kq_rest = ([('xk', i) for i in range(1, min(3, len(KC)))]
                       + [('wk', 1), ('wq', 1)]
                       + [('xk', i) for i in range(3, len(KC))])
            for j in range(max(SKT, len(kq_rest))):
                if j < SKT:
                    nc.sync.dma_start(out=xv_sb[j], in_=xv_d[j])
                if j < len(kq_rest):
                    kind, i = kq_rest[j]
                    if kind == 'xk':
                        nc.sync.dma_start(out=xk_sb[i], in_=xk_d[i])
                    elif kind == 'wk':
                        nc.sync.dma_start(out=wk_sb[1], in_=wk_d[1])
                    else:
                        nc.sync.dma_start(out=wq_sb[1], in_=wq_d[1])
            for i in range(1, NB):
                nc.sync.dma_start(out=xq_sb[i], in_=xq_d[i])
            nc.sync.dma_start(out=wo_sb, in_=wo_d)

            nc.vector.memset(ones_f, 1.0)
            nc.vector.tensor_copy(ones_h, ones_f)
            # pre-load the ScalarE exp table during the initial DMAs
            warm = constp.tile([128, 1], F32)
            nc.scalar.activation(warm, ones_f[:, 0:1], EXPF)


            # ---- emission helpers ----
            def emit_proj(which, pair, ci):
                """Q or K projection for one pair over one N-chunk."""
                w_sb, xt, pt, bcol, (c0, clen) = (
                    (wq_sb[pair], xq_sb[ci], qt_sb, pair, QBL[ci])
                    if which == 'q'
                    else (wk_sb[pair], xk_sb[ci], kt_sb, 2 + pair, KC[ci]))
                pp = ps_pj.tile([128, 512], F32, tag="pj",
                                name=f"pp_{which}{pair}_{c0}")
                for t in range(DT):
                    nc.tensor.matmul(
                        pp[:, :clen],
                        w_sb[:, t, :],
                        xt[:, t, :],
                        start=(t == 0), stop=(t == DT - 1))
                nc.vector.tensor_scalar_add(
                    pt[:, pair, c0:c0 + clen], pp[:, :clen],
                    qkb_sb[:, bcol:bcol + 1])

            def emit_vproj(s):
                pv = ps_acc.tile([128, 512], F32, tag="acc", name=f"pv{s}")
                for t in range(DT):
                    nc.tensor.matmul(
                        pv[:, 0:256],
                        xv_sb[s][:, t, :],
                        wv_sb[:, t, :],
                        start=(t == 0), stop=(t == DT - 1))
                nc.vector.tensor_add(v_sb[:, s, :], pv[:, 0:256], vb_bc)

            def emit_avz(pair, s, e2, op, zp, blen):
                for hh in range(2):
                    h = pair * 2 + hh
                    nc.tensor.matmul(
                        op[hh * 64:(hh + 1) * 64, :],
                        v_sb[:, s, h * 64:(h + 1) * 64],
                        e2[:, hh, :],
                        start=(s == 0), stop=(s == SKT - 1))
                for hh in range(2):
                    h = pair * 2 + hh
                    nc.tensor.matmul(
                        zp[32 * h:32 * h + 1, :],
                        ones_h[:, 0:1], e2[:, hh, :],
                        start=(s == 0), stop=(s == SKT - 1),
                        tile_position=(0, 32 * h))

            def emit_outproj(bi, tail=False):
                b0, blen = QBL[bi]
                for sqt in range(blen // 128):
                    q0 = b0 + sqt * 128
                    ob = work.tile([128, OUT], F16, tag="ob", bufs=3,
                                   name=f"ob{bi}_{sqt}")
                    for ch in range(2):
                        # In-stream outprojs use only the pj bank (the acc
                        # slots must stay available for the block
                        # accumulators).  Tail outprojs rotate over the
                        # banks the finished exp stream no longer needs,
                        # so consecutive chunks pipeline.
                        if tail:
                            pool, tg = [(ps_pj, "pj"), (ps_zp, "zp"),
                                        (ps_s2, "s2")][(sqt * 2 + ch) % 3]
                        else:
                            pool, tg = ps_pj, "pj"
                        po = pool.tile([128, 512], F32, tag=tg,
                                       name=f"po{bi}_{sqt}_{ch}")
                        for kt in range(2):
                            nc.tensor.matmul(
                                po,
                                ot_sb[:, kt, q0:q0 + 128],
                                wo_sb[:, kt, ch * 512:(ch + 1) * 512],
                                start=(kt == 0), stop=(kt == 1))
                        nc.vector.tensor_copy(
                            ob[:, ch * 512:(ch + 1) * 512], po)
                    nc.gpsimd.dma_start(out=outp[q0:q0 + 128, :], in_=ob)

            # ---- all projections up front (deps correct by construction;
            # the exp-critical chain outranks them via high_priority).
            # Priority order == need order of the exp stream.
            emit_proj('k', 0, 0)
            emit_proj('q', 0, 0)
            if len(KC) > 1:
                emit_proj('k', 0, 1)
            emit_proj('k', 1, 0)
            emit_proj('q', 1, 0)
            for ci in range(2, len(KC)):
                emit_proj('k', 0, ci)
            for ci in range(1, len(KC)):
                emit_proj('k', 1, ci)
            if NB > 1:
                emit_proj('q', 0, 1)
                emit_proj('q', 1, 1)
            for s in range(SKT):
                emit_vproj(s)
            for bi in range(2, NB):
                emit_proj('q', 0, bi)
                emit_proj('q', 1, bi)

            # ---- blocks, pair-major ----
            for bi, (b0, blen) in enumerate(QBL):
                zp = ps_zp.tile([128, blen], F32, tag="zp", name=f"zp{bi}")
                opsb = [None, None]
                for pair in range(2):
                    op = ps_acc.tile([128, blen], F32, tag="acc",
                                     name=f"op{bi}_{pair}")
                    prev = None
                    for s in range(SKT):
                        with tc.high_priority(offset=1_000_000):
                            st2 = ps_s2.tile([128, 2, 512], F32, tag="s2",
                                             name=f"st{bi}_{pair}_{s}")
                            for hh in range(2):
                                nc.tensor.matmul(
                                    st2[:, hh, :blen],
                                    kt_sb[hh * 64:(hh + 1) * 64, pair,
                                          s * 128:(s + 1) * 128],
                                    qt_sb[hh * 64:(hh + 1) * 64, pair,
                                          b0:b0 + blen],
                                    start=True, stop=True)
                            e2 = etile.tile([128, 2, blen], F16, tag="e",
                                            name=f"e{bi}_{pair}_{s}")
                            nc.scalar.activation(
                                e2, st2[:, :, :blen], EXPF,
                                bias=kbias_sb[:, s:s + 1], scale=SCALE)
                        if prev is not None:
                            with tc.high_priority(offset=500_000):
                                emit_avz(pair, prev[0], prev[1], op, zp, blen)
                        prev = (s, e2)
                    with tc.high_priority(offset=500_000):
                        emit_avz(pair, prev[0], prev[1], op, zp, blen)
                    osb = work.tile([128, blen], F32, tag=f"osb{pair}",
                                    bufs=2, name=f"osb{bi}_{pair}")
                    nc.vector.tensor_copy(osb, op)
                    opsb[pair] = osb

                # block end: Z -> 1/Z -> PE broadcast of 1/Z across the 64
                # head-dim partitions (ones ^T (1xK) matmul) -> normalize.
                # The muls read the broadcast straight from PSUM.
                nc.vector.tensor_copy(zq_sb[:, b0:b0 + blen], zp)
                nc.vector.reciprocal_approx_fast(
                    zinv_sb[:, b0:b0 + blen], zq_sb[:, b0:b0 + blen])
                for pair in range(2):
                    zbc = ps_acc.tile([128, blen], F32, tag="acc",
                                      name=f"zbc{bi}_{pair}")
                    for hh in range(2):
                        h = 2 * pair + hh
                        nc.tensor.matmul(
                            zbc[64 * hh:64 * (hh + 1), :],
                            ones_f[32 * h:32 * h + 1, 0:64],
                            zinv_sb[32 * h:32 * h + 1, b0:b0 + blen],
                            start=True, stop=True,
                            tile_position=(32 * h, 64 * hh))
                    for hh in range(2):
                        nc.vector.tensor_mul(
                            ot_sb[64 * hh:64 * (hh + 1), pair, b0:b0 + blen],
                            opsb[pair][64 * hh:64 * (hh + 1), :],
                            zbc[64 * hh:64 * (hh + 1), :])
                if 0 < bi <= NB - 2:
                    emit_outproj(bi - 1)

            if NB > 1:
                emit_outproj(NB - 2, tail=True)
            emit_outproj(NB - 1, tail=True)

    nc.compile()
    return nc


_NC_CACHE = {}


def _get_kernel(SQP, SKP):
    key = (SQP, SKP)
    if key not in _NC_CACHE:
        _NC_CACHE[key] = build_kernel(SQP, SKP)
    return _NC_CACHE[key]


def _ref_numpy(q, k, v, Wq, bq, Wk, bk, Wv, bv, Wo, bo, qm, vm):
    """Exact-reference fallback for degenerate masks (all-zero v_mask)."""
    qp = (q @ Wq + bq).reshape(S, H, HS)
    kp = (k @ Wk + bk).reshape(S, H, HS)
    vp = (v @ Wv + bv).reshape(S, H, HS)
    a = np.einsum('qhd,khd->hqk', qp, kp) / np.sqrt(HS)
    a = a - (1.0 - vm[None, None, :]) * 1e12
    a = a - a.max(-1, keepdims=True)
    e = np.exp(a)
    p = e / e.sum(-1, keepdims=True)
    o = np.einsum('hqk,khd->qhd', p, vp).reshape(S, H * HS)
    return (o @ Wo + bo) * qm[:, None]


def run(query, key, value, Wq, bq, Wk, bk, Wv, bv, Wo, bo, q_mask, v_mask,
        trace=False):
    query = np.asarray(query, np.float32)
    key = np.asarray(key, np.float32)
    value = np.asarray(value, np.float32)
    Wq, bq = np.asarray(Wq, np.float32), np.asarray(bq, np.float32)
    Wk, bk = np.asarray(Wk, np.float32), np.asarray(bk, np.float32)
    Wv, bv = np.asarray(Wv, np.float32), np.asarray(bv, np.float32)
    Wo, bo = np.asarray(Wo, np.float32), np.asarray(bo, np.float32)
    q_mask = np.asarray(q_mask)
    v_mask = np.asarray(v_mask)

    qidx = [np.nonzero(q_mask[b])[0] for b in range(B)]
    kidx = [np.nonzero(v_mask[b])[0] for b in range(B)]
    host_fallback = [len(kidx[b]) == 0 for b in range(B)]

    nq = max([128] + [len(i) for b, i in enumerate(qidx) if not host_fallback[b]])
    nk = max([128] + [len(i) for b, i in enumerate(kidx) if not host_fallback[b]])
    SQP = ((nq + 127) // 128) * 128
    SKP = ((nk + 127) // 128) * 128
    SKT = SKP // 128

    nc = _get_kernel(SQP, SKP)

    in_maps = []
    for c in range(NCORES):
        b, hg = c // 4, c % 4
        hc = slice(hg * HPC * HS, (hg + 1) * HPC * HS)  # this core's 256 head cols
        xq = np.zeros((SQP, D), np.float32)
        xk = np.zeros((SKP, D), np.float32)
        xv = np.zeros((SKP, D), np.float32)
        if not host_fallback[b]:
            xq[:len(qidx[b])] = query[b][qidx[b]]
            xk[:len(kidx[b])] = key[b][kidx[b]]
            xv[:len(kidx[b])] = value[b][kidx[b]]
        qkb = np.stack([bq[hc][:128], bq[hc][128:],
                        bk[hc][:128], bk[hc][128:]], axis=1)
        nkb = len(kidx[b]) if not host_fallback[b] else 0
        kbias = np.where(np.arange(SKP) < nkb, 0.0, KPAD_BIAS).astype(np.float32)
        xqT = xq.T.reshape(DT, 128, SQP).astype(np.float16)
        xkT = xk.T.reshape(DT, 128, SKP).astype(np.float16)
        xvT = xv.T.reshape(DT, 128, SKP).astype(np.float16)
        m = {}
        for s in range(SKT):
            m[f'xv{s}'] = np.ascontiguousarray(
                xvT[:, :, s * 128:(s + 1) * 128].transpose(1, 0, 2))
        for i, (b0, blen) in enumerate(_bblocks(SQP)):
            m[f'xq{i}'] = np.ascontiguousarray(
                xqT[:, :, b0:b0 + blen].transpose(1, 0, 2))
        off = 0
        i = 0
        while off < SKP:
            w = min(256, SKP - off)
            m[f'xk{i}'] = np.ascontiguousarray(
                xkT[:, :, off:off + w].transpose(1, 0, 2))
            off += w
            i += 1
        wqt = Wq[:, hc].reshape(DT, 128, 256).transpose(1, 0, 2).astype(np.float16)
        wkt = Wk[:, hc].reshape(DT, 128, 256).transpose(1, 0, 2).astype(np.float16)
        for p in range(2):
            m[f'wq{p}'] = np.ascontiguousarray(wqt[:, :, p * 128:(p + 1) * 128])
            m[f'wk{p}'] = np.ascontiguousarray(wkt[:, :, p * 128:(p + 1) * 128])
        m.update({
            'wv': np.ascontiguousarray(Wv[:, hc].reshape(DT, 128, 256).transpose(1, 0, 2)).astype(np.float16),
            'wo': np.ascontiguousarray(Wo[hc, :].reshape(2, 128, OUT).transpose(1, 0, 2)).astype(np.float16),
            'qkb': np.ascontiguousarray(qkb),
            'vb': np.ascontiguousarray(bv[hc].reshape(1, 256)),
            'kbias': np.ascontiguousarray(kbias.reshape(SKT, 128).T),
        })
        in_maps.append(m)

    res = bass_utils.run_bass_kernel_spmd(
        nc, in_maps, core_ids=list(range(NCORES)), trace=trace)

    out = np.zeros((B, S, OUT), np.float32)
    for b in range(B):
        if host_fallback[b]:
            out[b] = _ref_numpy(query[b], key[b], value[b], Wq, bq, Wk, bk,
                                Wv, bv, Wo, bo,
                                q_mask[b].astype(np.float32),
                                v_mask[b].astype(np.float32))
            continue
        acc = np.zeros((SQP, OUT), np.float32)
        for c in range(4 * b, 4 * b + 4):
            acc += res.results[c]['outp'].astype(np.float32)
        nqb = len(qidx[b])
        out[b][qidx[b]] = acc[:nqb] + bo
    return out, res


def kernel(**inputs):
    out, _ = run(**inputs)
    return out
